# revision 22
# baseline (speedup 1.0000x reference)
"""Tacotron2-style location-sensitive attention on 8 TRN2 NeuronCores.

Data-parallel over batch: 64 batches -> 8 per core. Weights replicated.

Math (per batch b):
  q_att = Wq @ query[b] + bq + bm                       [128]
  k     = memory[b] @ Wm.T                              [2048, 128]
  loc.T = WeffT.T @ X2[b]  where Weff = Wloc @ Wconv.reshape(32,62)
  s.T   = k.T + loc.T  (PSUM accumulation), tanh(s + q_att) via ACT bias
  e     = v . tanh_s  (bv dropped: constant shift cancels in softmax)
  w     = softmax(e)  (no max subtraction: |e| <= sum|v| ~ 9, safe in f32)
  ctx   = w @ memory[b]
Outputs: ctx [64, 512], w [64, 2048].

Hardware notes:
 - PE matmul instructions can carry only ONE semaphore wait in this
   toolchain, so all constants arrive via a single DMA (cpk), PSUM
   evacuations are split between DVE/ACT by t-group, and the emission
   order is arranged so recycle-waits are subsumed by earlier waits.
 - All heavy matmuls use float32r (1 col/cycle at N=512 vs 4 for f32).
"""

import os
import sys

sys.path.insert(0, "/opt/trn_rl_repo")

import numpy as np

from concourse import bass, bacc, tile, mybir
from concourse.bass_utils import run_bass_kernel_spmd

B, T = 64, 2048
D_LSTM, D_ENC, D_ATT, N_F, K = 1024, 512, 128, 32, 31
PAD = (K - 1) // 2
NCORES = 8
BPC = B // NCORES  # 8 batches per core
CK = 2 * K  # 62 im2col rows

f32 = mybir.dt.float32
f32r = mybir.dt.float32r
AF = mybir.ActivationFunctionType

bf16 = mybir.dt.bfloat16

# packed bf16 constants cpk [128, CPK_COLS]
OFF_WEFFT = 0  # rows 0..61, 128 cols
OFF_WQT = 128  # [d%128, dch*128+a] 1024 cols
OFF_QT = 1152  # [d%128, dch*8+b] 64 cols
OFF_V = 1216  # 1 col
OFF_WMT = 1217  # [e%128, eg*128+a] 512 cols
CPK_COLS = 1729
# f32 constants cf32 [128, CF_COLS]
OFF_BQM = 0  # 1 col
OFF_ID16 = 1  # rows 0..15, 16 cols
OFF_ONES16 = 17  # row 0, 16 cols
CF_COLS = 33


def build_nc():
    nc = bacc.Bacc(
        "TRN2", target_bir_lowering=False, debug=False, num_devices=NCORES
    )

    memn_d = nc.declare_dram_parameter("memn", [BPC, 128, 16, 512], bf16, isOutput=False)
    memt_d = nc.declare_dram_parameter("memt", [BPC, 128, 4, T], bf16, isOutput=False)
    x2_d = nc.declare_dram_parameter("x2", [BPC, CK, T], bf16, isOutput=False)
    cpk_d = nc.declare_dram_parameter("cpk", [128, CPK_COLS], bf16, isOutput=False)
    cf32_d = nc.declare_dram_parameter("cf32", [128, CF_COLS], f32, isOutput=False)
    ctx_d = nc.declare_dram_parameter("out_ctx", [BPC, D_ENC], f32, isOutput=True)
    w_d = nc.declare_dram_parameter("out_w", [BPC, T], f32, isOutput=True)

    with tile.TileContext(nc) as tc:
        with (
            tc.tile_pool(name="const", bufs=1) as constp,
            tc.tile_pool(name="memn", bufs=3) as memn_p,
            tc.tile_pool(name="memt", bufs=3) as memt_p,
            tc.tile_pool(name="x2p", bufs=2) as x2_p,
            tc.tile_pool(name="work", bufs=3) as work_p,
            tc.tile_pool(name="small", bufs=3) as small_p,
            tc.tile_pool(name="ps_s", bufs=3, space=bass.MemorySpace.PSUM) as ps_s,
            tc.tile_pool(name="ps_e", bufs=2, space=bass.MemorySpace.PSUM) as ps_e,
            tc.tile_pool(name="ps_wt", bufs=1, space=bass.MemorySpace.PSUM) as ps_wt,
            tc.tile_pool(name="ps_cq", bufs=1, space=bass.MemorySpace.PSUM) as ps_cq,
            tc.tile_pool(name="ps_rz", bufs=1, space=bass.MemorySpace.PSUM) as ps_rz,
        ):
            cpk = constp.tile([128, CPK_COLS], bf16, tag="cpk")
            nc.gpsimd.dma_start(cpk[:], cpk_d[:, :])
            cf32 = constp.tile([128, CF_COLS], f32, tag="cf32")
            nc.gpsimd.dma_start(cf32[:], cf32_d[:, :])

            def wmT(eg):
                return cpk[:, OFF_WMT + eg * 128 : OFF_WMT + (eg + 1) * 128]

            weffT = cpk[0:CK, OFF_WEFFT : OFF_WEFFT + 128]

            def wqT(dch):
                return cpk[:, OFF_WQT + dch * 128 : OFF_WQT + (dch + 1) * 128]

            def qTc(dch):
                return cpk[:, OFF_QT + dch * 8 : OFF_QT + (dch + 1) * 8]

            v_ap = cpk[:, OFF_V : OFF_V + 1]
            bqm_ap = cf32[:, OFF_BQM : OFF_BQM + 1]
            id16 = cf32[0:16, OFF_ID16 : OFF_ID16 + 16]
            ones16 = cf32[0:1, OFF_ONES16 : OFF_ONES16 + 16]

            # ---- q_att for all local batches: [a=128, BPC] ----
            qatt_ps = ps_cq.tile([128, BPC], f32, tag="cq_ps")
            for dch in range(8):
                nc.tensor.matmul(
                    qatt_ps[:], wqT(dch), qTc(dch), start=(dch == 0), stop=(dch == 7)
                )
            qatt_sb = constp.tile([128, BPC], f32, tag="qatt")
            nc.scalar.activation(qatt_sb[:], qatt_ps[:], AF.Identity, bias=bqm_ap)

            def stage_b(st):
                """Deferred per-batch tail: wT transpose, context, outputs."""
                b = st["b"]
                exp16_sb = st["exp16"]
                rz_sb = st["rz"]
                memn = st["memn"]
                wT_ps = ps_wt.tile([128, 16], f32, tag="wt_ps")
                nc.tensor.transpose(wT_ps[:], exp16_sb[:], id16)
                wT_sb = work_p.tile([128, 16], bf16, tag="wT")
                nc.scalar.copy(wT_sb[:], wT_ps[:])
                ctx_ps = ps_cq.tile([1, 512], f32, tag="cq_ps")
                for n in range(16):
                    nc.tensor.matmul(
                        ctx_ps[:],
                        wT_sb[:, n : n + 1],
                        memn[:, n, :],
                        start=(n == 0),
                        stop=(n == 15),
                    )
                ctx_sb = work_p.tile([1, 512], f32, tag="ctx_sb")
                nc.scalar.mul(ctx_sb[:], ctx_ps[:], rz_sb[0:1, 0:1])
                nc.scalar.dma_start(ctx_d[b : b + 1, :], ctx_sb[:])
                rz16_ps = ps_rz.tile([16, 1], f32, tag="rz_ps")
                nc.tensor.matmul(rz16_ps[:], ones16, rz_sb[:], start=True, stop=True)
                rz16_sb = small_p.tile([16, 1], f32, tag="rz16")
                nc.scalar.copy(rz16_sb[:], rz16_ps[:])
                w_sb = work_p.tile([16, 128], f32, tag="w_sb")
                nc.scalar.mul(w_sb[:], exp16_sb[:], rz16_sb[:, 0:1])
                nc.scalar.dma_start(w_d[b : b + 1, :], w_sb[:])

            prev = None
            for b in range(BPC):
                # both layouts pre-transposed on host, bf16
                memn = memn_p.tile([128, 16, 512], bf16, tag="memn")
                nc.gpsimd.dma_start(
                    memn[:], memn_d[b : b + 1].rearrange("o p n d -> p (o n) d")
                )
                memt = memt_p.tile([128, 4, T], bf16, tag="memt")
                nc.sync.dma_start(
                    memt[:], memt_d[b : b + 1].rearrange("o p g t -> p (o g) t")
                )
                x2 = x2_p.tile([CK, T], bf16, tag="x2")
                nc.scalar.dma_start(
                    x2[:], x2_d[b : b + 1, :, :].rearrange("o c t -> c (o t)")
                )

                exp_sb = work_p.tile([1, T], f32, tag="exp")
                zs_sb = small_p.tile([1, 4], f32, tag="zs")
                for tg in range(4):
                    tsl = slice(tg * 512, (tg + 1) * 512)
                    # energies for this t-group
                    s_ps = ps_s.tile([128, 512], f32, tag="s_ps")
                    for eg in range(4):
                        nc.tensor.matmul(
                            s_ps[:],
                            wmT(eg),
                            memt[:, eg, tsl],
                            start=(eg == 0),
                            stop=False,
                        )
                    nc.tensor.matmul(
                        s_ps[:], weffT, x2[:, tsl], start=False, stop=True
                    )
                    th_sb = work_p.tile([128, 512], bf16, tag="tanh")
                    nc.scalar.activation(
                        th_sb[:], s_ps[:], AF.Tanh, bias=qatt_sb[:, b : b + 1]
                    )
                    e_ps = ps_e.tile([1, 512], f32, tag="e_ps")
                    nc.tensor.matmul(e_ps[:], v_ap, th_sb[:], start=True, stop=True)
                    nc.scalar.activation(
                        exp_sb[0:1, tsl],
                        e_ps[:],
                        AF.Exp,
                        accum_out=zs_sb[0:1, tg : tg + 1],
                    )

                # softmax denominator (DVE)
                z_sb = small_p.tile([1, 1], f32, tag="z")
                nc.vector.tensor_reduce(
                    z_sb[:], zs_sb[:], axis=mybir.AxisListType.X, op=mybir.AluOpType.add
                )
                rz_sb = small_p.tile([1, 1], f32, tag="rz")
                nc.vector.reciprocal(rz_sb[:], z_sb[:])
                # reshape [1,2048] -> [16,128] via DMA (idle sync ring)
                exp16_sb = work_p.tile([16, 128], f32, tag="exp16")
                nc.sync.dma_start(exp16_sb[:], exp_sb[:])

                if prev is not None:
                    stage_b(prev)
                prev = {"b": b, "exp16": exp16_sb, "rz": rz_sb, "memn": memn}
            stage_b(prev)

    nc.compile()
    return nc


def host_prep(inputs):
    """Precompute constant-folded weights and per-core input shards."""
    query = np.asarray(inputs["query"], dtype=np.float32)  # [B,1,D_LSTM]
    memory = np.ascontiguousarray(np.asarray(inputs["memory"], dtype=np.float32))
    aw = np.asarray(inputs["attention_weights_cat"], dtype=np.float32)  # [B,2,T]
    Wq = np.asarray(inputs["Wq"], dtype=np.float32)
    bq = np.asarray(inputs["bq"], dtype=np.float32)
    Wm = np.asarray(inputs["Wm"], dtype=np.float32)
    bm = np.asarray(inputs["bm"], dtype=np.float32)
    Wconv = np.asarray(inputs["Wconv"], dtype=np.float32)  # [N_F, 2, K]
    Wloc = np.asarray(inputs["Wloc"], dtype=np.float32)  # [D_ATT, N_F]
    Wv = np.asarray(inputs["Wv"], dtype=np.float32)  # [1, D_ATT]

    awpad = np.zeros((B, 2, T + 2 * PAD), dtype=np.float32)
    awpad[:, :, PAD : PAD + T] = aw
    # X2[b, c*K+k, t] = awpad[b, c, t+k]
    x2 = np.lib.stride_tricks.sliding_window_view(awpad, T, axis=2)
    x2 = np.ascontiguousarray(x2.reshape(B, CK, T))

    weff = Wloc @ Wconv.reshape(N_F, CK)  # [D_ATT, CK]

    import ml_dtypes

    x2 = x2.astype(ml_dtypes.bfloat16)

    base = np.zeros((128, CPK_COLS), dtype=np.float32)
    base[:, OFF_WMT : OFF_WMT + 512] = (
        Wm.T.reshape(4, 128, D_ATT).transpose(1, 0, 2).reshape(128, 512)
    )
    base[0:CK, OFF_WEFFT : OFF_WEFFT + 128] = weff.T
    base[:, OFF_WQT : OFF_WQT + 1024] = (
        Wq.T.reshape(8, 128, D_ATT).transpose(1, 0, 2).reshape(128, 1024)
    )
    base[:, OFF_V] = Wv.reshape(D_ATT)

    memb = memory.astype(ml_dtypes.bfloat16)
    # memn[b, p, n, d] = mem[b, n*128+p, d]
    memn_h = np.ascontiguousarray(
        memb.reshape(B, 16, 128, D_ENC).transpose(0, 2, 1, 3)
    )
    # memt[b, p, g, t] = mem[b, t, g*128+p]
    memt_h = np.ascontiguousarray(
        memb.transpose(0, 2, 1).reshape(B, 4, 128, T).transpose(0, 2, 1, 3)
    )

    cf = np.zeros((128, CF_COLS), dtype=np.float32)
    cf[:, OFF_BQM] = bq + bm
    cf[0:16, OFF_ID16 : OFF_ID16 + 16] = np.eye(16, dtype=np.float32)
    cf[0, OFF_ONES16 : OFF_ONES16 + 16] = 1.0

    in_maps = []
    for i in range(NCORES):
        sl = slice(i * BPC, (i + 1) * BPC)
        cpk = base.copy()
        qTi = query[sl, 0, :].T  # [D_LSTM, BPC]
        cpk[:, OFF_QT : OFF_QT + 64] = (
            qTi.reshape(8, 128, BPC).transpose(1, 0, 2).reshape(128, 64)
        )
        in_maps.append(
            {
                "memn": memn_h[sl],
                "memt": memt_h[sl],
                "x2": x2[sl],
                "cpk": cpk.astype(ml_dtypes.bfloat16),
                "cf32": cf,
            }
        )
    return in_maps


_NC_CACHE = {}


def _get_nc():
    if "nc" not in _NC_CACHE:
        _NC_CACHE["nc"] = build_nc()
    return _NC_CACHE["nc"]


def run(inputs, trace=False, **kw):
    nc = _get_nc()
    in_maps = host_prep(inputs)
    res = run_bass_kernel_spmd(nc, in_maps, list(range(NCORES)), trace=trace, **kw)
    outs = res.results
    ctx = np.concatenate([np.asarray(outs[i]["out_ctx"]) for i in range(NCORES)], axis=0)
    w = np.concatenate([np.asarray(outs[i]["out_w"]) for i in range(NCORES)], axis=0)
    return (ctx, w), res


def kernel(**inputs):
    (ctx, w), _ = run(inputs, trace=False)
    return ctx, w


# revision 23
# speedup vs baseline: 1.0709x; 1.0709x over previous
"""Tacotron2-style location-sensitive attention on 8 TRN2 NeuronCores.

Data-parallel over batch: 64 batches -> 8 per core. Weights replicated.

Math (per batch b):
  q_att = Wq @ query[b] + bq + bm                       [128]
  k     = memory[b] @ Wm.T                              [2048, 128]
  loc.T = WeffT.T @ X2[b]  where Weff = Wloc @ Wconv.reshape(32,62)
  s.T   = k.T + loc.T  (PSUM accumulation), tanh(s + q_att) via ACT bias
  e     = v . tanh_s  (bv dropped: constant shift cancels in softmax)
  w     = softmax(e)  (no max subtraction: |e| <= sum|v| ~ 9, safe in f32)
  ctx   = w @ memory[b]
Outputs: ctx [64, 512], w [64, 2048].

Hardware notes:
 - PE matmul instructions can carry only ONE semaphore wait in this
   toolchain, so all constants arrive via a single DMA (cpk), PSUM
   evacuations are split between DVE/ACT by t-group, and the emission
   order is arranged so recycle-waits are subsumed by earlier waits.
 - All heavy matmuls use float32r (1 col/cycle at N=512 vs 4 for f32).
"""

import os
import sys

sys.path.insert(0, "/opt/trn_rl_repo")

import numpy as np

from concourse import bass, bacc, tile, mybir
from concourse.bass_utils import run_bass_kernel_spmd

B, T = 64, 2048
D_LSTM, D_ENC, D_ATT, N_F, K = 1024, 512, 128, 32, 31
PAD = (K - 1) // 2
NCORES = 8
BPC = B // NCORES  # 8 batches per core
CK = 2 * K  # 62 im2col rows

f32 = mybir.dt.float32
f32r = mybir.dt.float32r
AF = mybir.ActivationFunctionType

bf16 = mybir.dt.bfloat16

# packed bf16 constants cpk [128, CPK_COLS]
OFF_WEFFT = 0  # rows 0..61, 128 cols
OFF_WQT = 128  # [d%128, dch*128+a] 1024 cols
OFF_QT = 1152  # [d%128, dch*8+b] 64 cols
OFF_V = 1216  # 1 col
OFF_WMT = 1217  # [e%128, eg*128+a] 512 cols
CPK_COLS = 1729
# f32 constants cf32 [128, CF_COLS]
OFF_BQM = 0  # 1 col
OFF_ID16 = 1  # rows 0..15, 16 cols
OFF_ONES16 = 17  # row 0, 16 cols
CF_COLS = 33


def build_nc():
    nc = bacc.Bacc(
        "TRN2", target_bir_lowering=False, debug=False, num_devices=NCORES
    )

    memn_d = nc.declare_dram_parameter("memn", [BPC, 128, 16, 512], bf16, isOutput=False)
    memt_d = nc.declare_dram_parameter("memt", [BPC, 128, 4, T], bf16, isOutput=False)
    x2_d = nc.declare_dram_parameter("x2", [BPC, CK, T], bf16, isOutput=False)
    cpk_d = nc.declare_dram_parameter("cpk", [128, CPK_COLS], bf16, isOutput=False)
    cf32_d = nc.declare_dram_parameter("cf32", [128, CF_COLS], f32, isOutput=False)
    ctx_d = nc.declare_dram_parameter("out_ctx", [BPC, D_ENC], f32, isOutput=True)
    w_d = nc.declare_dram_parameter("out_w", [BPC, T], f32, isOutput=True)

    with tile.TileContext(nc) as tc:
        with (
            tc.tile_pool(name="const", bufs=1) as constp,
            tc.tile_pool(name="memn", bufs=3) as memn_p,
            tc.tile_pool(name="memt", bufs=3) as memt_p,
            tc.tile_pool(name="x2p", bufs=2) as x2_p,
            tc.tile_pool(name="work", bufs=3) as work_p,
            tc.tile_pool(name="small", bufs=3) as small_p,
            tc.tile_pool(name="ps_s", bufs=3, space=bass.MemorySpace.PSUM) as ps_s,
            tc.tile_pool(name="ps_e", bufs=2, space=bass.MemorySpace.PSUM) as ps_e,
            tc.tile_pool(name="ps_wt", bufs=1, space=bass.MemorySpace.PSUM) as ps_wt,
            tc.tile_pool(name="ps_cq", bufs=1, space=bass.MemorySpace.PSUM) as ps_cq,
            tc.tile_pool(name="ps_rz", bufs=1, space=bass.MemorySpace.PSUM) as ps_rz,
        ):
            cpk = constp.tile([128, CPK_COLS], bf16, tag="cpk")
            nc.gpsimd.dma_start(cpk[:], cpk_d[:, :])
            cf32 = constp.tile([128, CF_COLS], f32, tag="cf32")
            nc.gpsimd.dma_start(cf32[:], cf32_d[:, :])

            def wmT(eg):
                return cpk[:, OFF_WMT + eg * 128 : OFF_WMT + (eg + 1) * 128]

            weffT = cpk[0:CK, OFF_WEFFT : OFF_WEFFT + 128]

            def wqT(dch):
                return cpk[:, OFF_WQT + dch * 128 : OFF_WQT + (dch + 1) * 128]

            def qTc(dch):
                return cpk[:, OFF_QT + dch * 8 : OFF_QT + (dch + 1) * 8]

            v_ap = cpk[:, OFF_V : OFF_V + 1]
            bqm_ap = cf32[:, OFF_BQM : OFF_BQM + 1]
            id16 = cf32[0:16, OFF_ID16 : OFF_ID16 + 16]
            ones16 = cf32[0:1, OFF_ONES16 : OFF_ONES16 + 16]

            # ---- q_att for all local batches: [a=128, BPC] ----
            qatt_ps = ps_cq.tile([128, BPC], f32, tag="cq_ps")
            for dch in range(8):
                nc.tensor.matmul(
                    qatt_ps[:], wqT(dch), qTc(dch), start=(dch == 0), stop=(dch == 7)
                )
            qatt_sb = constp.tile([128, BPC], f32, tag="qatt")
            nc.scalar.activation(qatt_sb[:], qatt_ps[:], AF.Identity, bias=bqm_ap)

            def stage_b(st):
                """Deferred per-batch tail: wT transpose, context, outputs."""
                b = st["b"]
                exp16_sb = st["exp16"]
                rz_sb = st["rz"]
                memn = st["memn"]
                wT_ps = ps_wt.tile([128, 16], f32, tag="wt_ps")
                nc.tensor.transpose(wT_ps[:], exp16_sb[:], id16)
                wT_sb = work_p.tile([128, 16], bf16, tag="wT")
                nc.scalar.copy(wT_sb[:], wT_ps[:])
                ctx_ps = ps_cq.tile([1, 512], f32, tag="cq_ps")
                for n in range(16):
                    nc.tensor.matmul(
                        ctx_ps[:],
                        wT_sb[:, n : n + 1],
                        memn[:, n, :],
                        start=(n == 0),
                        stop=(n == 15),
                    )
                ctx_sb = work_p.tile([1, 512], f32, tag="ctx_sb")
                nc.scalar.mul(ctx_sb[:], ctx_ps[:], rz_sb[0:1, 0:1])
                nc.scalar.dma_start(ctx_d[b : b + 1, :], ctx_sb[:])
                rz16_ps = ps_rz.tile([16, 1], f32, tag="rz_ps")
                nc.tensor.matmul(rz16_ps[:], ones16, rz_sb[:], start=True, stop=True)
                rz16_sb = small_p.tile([16, 1], f32, tag="rz16")
                nc.scalar.copy(rz16_sb[:], rz16_ps[:])
                w_sb = work_p.tile([16, 128], f32, tag="w_sb")
                nc.scalar.mul(w_sb[:], exp16_sb[:], rz16_sb[:, 0:1])
                nc.scalar.dma_start(w_d[b : b + 1, :], w_sb[:])

            prev = None
            for b in range(BPC):
                # both layouts pre-transposed on host, bf16
                memn = memn_p.tile([128, 16, 512], bf16, tag="memn")
                nc.gpsimd.dma_start(
                    memn[:], memn_d[b : b + 1].rearrange("o p n d -> p (o n) d")
                )
                memt = memt_p.tile([128, 4, T], bf16, tag="memt")
                nc.sync.dma_start(
                    memt[:], memt_d[b : b + 1].rearrange("o p g t -> p (o g) t")
                )
                x2 = x2_p.tile([CK, T], bf16, tag="x2")
                nc.scalar.dma_start(
                    x2[:], x2_d[b : b + 1, :, :].rearrange("o c t -> c (o t)")
                )

                exp_sb = work_p.tile([1, T], f32, tag="exp")
                zs_sb = small_p.tile([1, 4], f32, tag="zs")
                for tg in range(4):
                    tsl = slice(tg * 512, (tg + 1) * 512)
                    # energies for this t-group
                    s_ps = ps_s.tile([128, 512], f32, tag="s_ps")
                    for eg in range(4):
                        nc.tensor.matmul(
                            s_ps[:],
                            wmT(eg),
                            memt[:, eg, tsl],
                            start=(eg == 0),
                            stop=False,
                        )
                    nc.tensor.matmul(
                        s_ps[:], weffT, x2[:, tsl], start=False, stop=True
                    )
                    th_sb = work_p.tile([128, 512], bf16, tag="tanh")
                    nc.scalar.activation(
                        th_sb[:], s_ps[:], AF.Tanh, bias=qatt_sb[:, b : b + 1]
                    )
                    e_ps = ps_e.tile([1, 512], f32, tag="e_ps")
                    nc.tensor.matmul(e_ps[:], v_ap, th_sb[:], start=True, stop=True)
                    nc.scalar.activation(
                        exp_sb[0:1, tsl],
                        e_ps[:],
                        AF.Exp,
                        accum_out=zs_sb[0:1, tg : tg + 1],
                    )

                # softmax denominator (DVE)
                z_sb = small_p.tile([1, 1], f32, tag="z")
                nc.vector.tensor_reduce(
                    z_sb[:], zs_sb[:], axis=mybir.AxisListType.X, op=mybir.AluOpType.add
                )
                rz_sb = small_p.tile([1, 1], f32, tag="rz")
                nc.vector.reciprocal(rz_sb[:], z_sb[:])
                # reshape [1,2048] -> [16,128] via DMA (idle sync ring)
                exp16_sb = work_p.tile([16, 128], f32, tag="exp16")
                nc.gpsimd.dma_start(exp16_sb[:], exp_sb[:])

                if prev is not None:
                    stage_b(prev)
                prev = {"b": b, "exp16": exp16_sb, "rz": rz_sb, "memn": memn}
            stage_b(prev)

    nc.compile()
    return nc


def host_prep(inputs):
    """Precompute constant-folded weights and per-core input shards."""
    query = np.asarray(inputs["query"], dtype=np.float32)  # [B,1,D_LSTM]
    memory = np.ascontiguousarray(np.asarray(inputs["memory"], dtype=np.float32))
    aw = np.asarray(inputs["attention_weights_cat"], dtype=np.float32)  # [B,2,T]
    Wq = np.asarray(inputs["Wq"], dtype=np.float32)
    bq = np.asarray(inputs["bq"], dtype=np.float32)
    Wm = np.asarray(inputs["Wm"], dtype=np.float32)
    bm = np.asarray(inputs["bm"], dtype=np.float32)
    Wconv = np.asarray(inputs["Wconv"], dtype=np.float32)  # [N_F, 2, K]
    Wloc = np.asarray(inputs["Wloc"], dtype=np.float32)  # [D_ATT, N_F]
    Wv = np.asarray(inputs["Wv"], dtype=np.float32)  # [1, D_ATT]

    awpad = np.zeros((B, 2, T + 2 * PAD), dtype=np.float32)
    awpad[:, :, PAD : PAD + T] = aw
    # X2[b, c*K+k, t] = awpad[b, c, t+k]
    x2 = np.lib.stride_tricks.sliding_window_view(awpad, T, axis=2)
    x2 = np.ascontiguousarray(x2.reshape(B, CK, T))

    weff = Wloc @ Wconv.reshape(N_F, CK)  # [D_ATT, CK]

    import ml_dtypes

    x2 = x2.astype(ml_dtypes.bfloat16)

    base = np.zeros((128, CPK_COLS), dtype=np.float32)
    base[:, OFF_WMT : OFF_WMT + 512] = (
        Wm.T.reshape(4, 128, D_ATT).transpose(1, 0, 2).reshape(128, 512)
    )
    base[0:CK, OFF_WEFFT : OFF_WEFFT + 128] = weff.T
    base[:, OFF_WQT : OFF_WQT + 1024] = (
        Wq.T.reshape(8, 128, D_ATT).transpose(1, 0, 2).reshape(128, 1024)
    )
    base[:, OFF_V] = Wv.reshape(D_ATT)

    memb = memory.astype(ml_dtypes.bfloat16)
    # memn[b, p, n, d] = mem[b, n*128+p, d]
    memn_h = np.ascontiguousarray(
        memb.reshape(B, 16, 128, D_ENC).transpose(0, 2, 1, 3)
    )
    # memt[b, p, g, t] = mem[b, t, g*128+p]
    memt_h = np.ascontiguousarray(
        memb.transpose(0, 2, 1).reshape(B, 4, 128, T).transpose(0, 2, 1, 3)
    )

    cf = np.zeros((128, CF_COLS), dtype=np.float32)
    cf[:, OFF_BQM] = bq + bm
    cf[0:16, OFF_ID16 : OFF_ID16 + 16] = np.eye(16, dtype=np.float32)
    cf[0, OFF_ONES16 : OFF_ONES16 + 16] = 1.0

    in_maps = []
    for i in range(NCORES):
        sl = slice(i * BPC, (i + 1) * BPC)
        cpk = base.copy()
        qTi = query[sl, 0, :].T  # [D_LSTM, BPC]
        cpk[:, OFF_QT : OFF_QT + 64] = (
            qTi.reshape(8, 128, BPC).transpose(1, 0, 2).reshape(128, 64)
        )
        in_maps.append(
            {
                "memn": memn_h[sl],
                "memt": memt_h[sl],
                "x2": x2[sl],
                "cpk": cpk.astype(ml_dtypes.bfloat16),
                "cf32": cf,
            }
        )
    return in_maps


_NC_CACHE = {}


def _get_nc():
    if "nc" not in _NC_CACHE:
        _NC_CACHE["nc"] = build_nc()
    return _NC_CACHE["nc"]


def run(inputs, trace=False, **kw):
    nc = _get_nc()
    in_maps = host_prep(inputs)
    res = run_bass_kernel_spmd(nc, in_maps, list(range(NCORES)), trace=trace, **kw)
    outs = res.results
    ctx = np.concatenate([np.asarray(outs[i]["out_ctx"]) for i in range(NCORES)], axis=0)
    w = np.concatenate([np.asarray(outs[i]["out_w"]) for i in range(NCORES)], axis=0)
    return (ctx, w), res


def kernel(**inputs):
    (ctx, w), _ = run(inputs, trace=False)
    return ctx, w


# revision 24
# speedup vs baseline: 1.1353x; 1.0602x over previous
"""Tacotron2-style location-sensitive attention on 8 TRN2 NeuronCores.

Data-parallel over batch: 64 batches -> 8 per core. Weights replicated.

Math (per batch b):
  q_att = Wq @ query[b] + bq + bm                       [128]
  k     = memory[b] @ Wm.T                              [2048, 128]
  loc.T = WeffT.T @ X2[b]  where Weff = Wloc @ Wconv.reshape(32,62)
  s.T   = k.T + loc.T  (PSUM accumulation), tanh(s + q_att) via ACT bias
  e     = v . tanh_s  (bv dropped: constant shift cancels in softmax)
  w     = softmax(e)  (no max subtraction: |e| <= sum|v| ~ 9, safe in f32)
  ctx   = w @ memory[b]
Outputs: ctx [64, 512], w [64, 2048].

Hardware notes:
 - PE matmul instructions can carry only ONE semaphore wait in this
   toolchain, so all constants arrive via a single DMA (cpk), PSUM
   evacuations are split between DVE/ACT by t-group, and the emission
   order is arranged so recycle-waits are subsumed by earlier waits.
 - All heavy matmuls use float32r (1 col/cycle at N=512 vs 4 for f32).
"""

import os
import sys

sys.path.insert(0, "/opt/trn_rl_repo")

import numpy as np

from concourse import bass, bacc, tile, mybir
from concourse.bass_utils import run_bass_kernel_spmd

B, T = 64, 2048
D_LSTM, D_ENC, D_ATT, N_F, K = 1024, 512, 128, 32, 31
PAD = (K - 1) // 2
NCORES = 8
BPC = B // NCORES  # 8 batches per core
CK = 2 * K  # 62 im2col rows

f32 = mybir.dt.float32
f32r = mybir.dt.float32r
AF = mybir.ActivationFunctionType

bf16 = mybir.dt.bfloat16

# packed bf16 constants cpk [128, CPK_COLS]
OFF_WEFFT = 0  # rows 0..61, 128 cols
OFF_WQT = 128  # [d%128, dch*128+a] 1024 cols
OFF_QT = 1152  # [d%128, dch*8+b] 64 cols
OFF_V = 1216  # 1 col
OFF_WMT = 1217  # [e%128, eg*128+a] 512 cols
CPK_COLS = 1729
# f32 constants cf32 [128, CF_COLS]
OFF_BQM = 0  # 1 col
OFF_ID16 = 1  # rows 0..15, 16 cols
OFF_ONES16 = 17  # row 0, 16 cols
CF_COLS = 33


def build_nc():
    nc = bacc.Bacc(
        "TRN2", target_bir_lowering=False, debug=False, num_devices=NCORES
    )

    mem2_d = nc.declare_dram_parameter("mem2", [BPC, 128, 16384], bf16, isOutput=False)
    x2_d = nc.declare_dram_parameter("x2", [BPC, CK, T], bf16, isOutput=False)
    cpk_d = nc.declare_dram_parameter("cpk", [128, CPK_COLS], bf16, isOutput=False)
    cf32_d = nc.declare_dram_parameter("cf32", [128, CF_COLS], f32, isOutput=False)
    ctx_d = nc.declare_dram_parameter("out_ctx", [BPC, D_ENC], f32, isOutput=True)
    w_d = nc.declare_dram_parameter("out_w", [BPC, T], f32, isOutput=True)

    with tile.TileContext(nc) as tc:
        with (
            tc.tile_pool(name="const", bufs=1) as constp,
            tc.tile_pool(name="memn", bufs=3) as memn_p,
            tc.tile_pool(name="x2p", bufs=2) as x2_p,
            tc.tile_pool(name="work", bufs=3) as work_p,
            tc.tile_pool(name="small", bufs=3) as small_p,
            tc.tile_pool(name="ps_s", bufs=3, space=bass.MemorySpace.PSUM) as ps_s,
            tc.tile_pool(name="ps_e", bufs=2, space=bass.MemorySpace.PSUM) as ps_e,
            tc.tile_pool(name="ps_wt", bufs=1, space=bass.MemorySpace.PSUM) as ps_wt,
            tc.tile_pool(name="ps_cq", bufs=1, space=bass.MemorySpace.PSUM) as ps_cq,
            tc.tile_pool(name="ps_rz", bufs=1, space=bass.MemorySpace.PSUM) as ps_rz,
        ):
            cpk = constp.tile([128, CPK_COLS], bf16, tag="cpk")
            nc.gpsimd.dma_start(cpk[:], cpk_d[:, :])
            cf32 = constp.tile([128, CF_COLS], f32, tag="cf32")
            nc.gpsimd.dma_start(cf32[:], cf32_d[:, :])

            def wmT(eg):
                return cpk[:, OFF_WMT + eg * 128 : OFF_WMT + (eg + 1) * 128]

            weffT = cpk[0:CK, OFF_WEFFT : OFF_WEFFT + 128]

            def wqT(dch):
                return cpk[:, OFF_WQT + dch * 128 : OFF_WQT + (dch + 1) * 128]

            def qTc(dch):
                return cpk[:, OFF_QT + dch * 8 : OFF_QT + (dch + 1) * 8]

            v_ap = cpk[:, OFF_V : OFF_V + 1]
            bqm_ap = cf32[:, OFF_BQM : OFF_BQM + 1]
            id16 = cf32[0:16, OFF_ID16 : OFF_ID16 + 16]
            ones16 = cf32[0:1, OFF_ONES16 : OFF_ONES16 + 16]

            # ---- q_att for all local batches: [a=128, BPC] ----
            qatt_ps = ps_cq.tile([128, BPC], f32, tag="cq_ps")
            for dch in range(8):
                nc.tensor.matmul(
                    qatt_ps[:], wqT(dch), qTc(dch), start=(dch == 0), stop=(dch == 7)
                )
            qatt_sb = constp.tile([128, BPC], f32, tag="qatt")
            nc.scalar.activation(qatt_sb[:], qatt_ps[:], AF.Identity, bias=bqm_ap)

            def stage_b(st):
                """Deferred per-batch tail: wT transpose, context, outputs."""
                b = st["b"]
                exp16_sb = st["exp16"]
                rz_sb = st["rz"]
                memn = st["memn"]
                wT_ps = ps_wt.tile([128, 16], f32, tag="wt_ps")
                nc.tensor.transpose(wT_ps[:], exp16_sb[:], id16)
                wT_sb = work_p.tile([128, 16], bf16, tag="wT")
                nc.scalar.copy(wT_sb[:], wT_ps[:])
                ctx_ps = ps_cq.tile([1, 512], f32, tag="cq_ps")
                for n in range(16):
                    nc.tensor.matmul(
                        ctx_ps[:],
                        wT_sb[:, n : n + 1],
                        memn[:, n, :],
                        start=(n == 0),
                        stop=(n == 15),
                    )
                ctx_sb = work_p.tile([1, 512], f32, tag="ctx_sb")
                nc.scalar.mul(ctx_sb[:], ctx_ps[:], rz_sb[0:1, 0:1])
                nc.scalar.dma_start(ctx_d[b : b + 1, :], ctx_sb[:])
                rz16_ps = ps_rz.tile([16, 1], f32, tag="rz_ps")
                nc.tensor.matmul(rz16_ps[:], ones16, rz_sb[:], start=True, stop=True)
                rz16_sb = small_p.tile([16, 1], f32, tag="rz16")
                nc.scalar.copy(rz16_sb[:], rz16_ps[:])
                w_sb = work_p.tile([16, 128], f32, tag="w_sb")
                nc.scalar.mul(w_sb[:], exp16_sb[:], rz16_sb[:, 0:1])
                nc.scalar.dma_start(w_d[b : b + 1, :], w_sb[:])

            prev = None
            for b in range(BPC):
                # both layouts pre-transposed on host, bf16, one DMA:
                # cols 0:8192 = natural [n, d], cols 8192:16384 = transposed [g, t]
                mem2 = memn_p.tile([128, 16384], bf16, tag="memn")
                nc.gpsimd.dma_start(
                    mem2[:], mem2_d[b : b + 1].rearrange("o p c -> p (o c)")
                )
                memn = mem2[:, 0:8192].rearrange("p (n d) -> p n d", n=16)
                memt = mem2[:, 8192:16384].rearrange("p (g t) -> p g t", g=4)
                x2 = x2_p.tile([CK, T], bf16, tag="x2")
                nc.scalar.dma_start(
                    x2[:], x2_d[b : b + 1, :, :].rearrange("o c t -> c (o t)")
                )

                exp_sb = work_p.tile([1, T], f32, tag="exp")
                zs_sb = small_p.tile([1, 4], f32, tag="zs")
                for tg in range(4):
                    tsl = slice(tg * 512, (tg + 1) * 512)
                    # energies for this t-group
                    s_ps = ps_s.tile([128, 512], f32, tag="s_ps")
                    for eg in range(4):
                        nc.tensor.matmul(
                            s_ps[:],
                            wmT(eg),
                            memt[:, eg, tsl],
                            start=(eg == 0),
                            stop=False,
                        )
                    nc.tensor.matmul(
                        s_ps[:], weffT, x2[:, tsl], start=False, stop=True
                    )
                    th_sb = work_p.tile([128, 512], bf16, tag="tanh")
                    nc.scalar.activation(
                        th_sb[:], s_ps[:], AF.Tanh, bias=qatt_sb[:, b : b + 1]
                    )
                    e_ps = ps_e.tile([1, 512], f32, tag="e_ps")
                    nc.tensor.matmul(e_ps[:], v_ap, th_sb[:], start=True, stop=True)
                    nc.scalar.activation(
                        exp_sb[0:1, tsl],
                        e_ps[:],
                        AF.Exp,
                        accum_out=zs_sb[0:1, tg : tg + 1],
                    )

                # softmax denominator (DVE)
                z_sb = small_p.tile([1, 1], f32, tag="z")
                nc.vector.tensor_reduce(
                    z_sb[:], zs_sb[:], axis=mybir.AxisListType.X, op=mybir.AluOpType.add
                )
                rz_sb = small_p.tile([1, 1], f32, tag="rz")
                nc.vector.reciprocal(rz_sb[:], z_sb[:])
                # reshape [1,2048] -> [16,128] via DMA (idle sync ring)
                exp16_sb = work_p.tile([16, 128], f32, tag="exp16")
                nc.sync.dma_start(exp16_sb[:], exp_sb[:])

                if prev is not None:
                    stage_b(prev)
                prev = {"b": b, "exp16": exp16_sb, "rz": rz_sb, "memn": memn}
            stage_b(prev)

    nc.compile()
    return nc


def host_prep(inputs):
    """Precompute constant-folded weights and per-core input shards."""
    query = np.asarray(inputs["query"], dtype=np.float32)  # [B,1,D_LSTM]
    memory = np.ascontiguousarray(np.asarray(inputs["memory"], dtype=np.float32))
    aw = np.asarray(inputs["attention_weights_cat"], dtype=np.float32)  # [B,2,T]
    Wq = np.asarray(inputs["Wq"], dtype=np.float32)
    bq = np.asarray(inputs["bq"], dtype=np.float32)
    Wm = np.asarray(inputs["Wm"], dtype=np.float32)
    bm = np.asarray(inputs["bm"], dtype=np.float32)
    Wconv = np.asarray(inputs["Wconv"], dtype=np.float32)  # [N_F, 2, K]
    Wloc = np.asarray(inputs["Wloc"], dtype=np.float32)  # [D_ATT, N_F]
    Wv = np.asarray(inputs["Wv"], dtype=np.float32)  # [1, D_ATT]

    awpad = np.zeros((B, 2, T + 2 * PAD), dtype=np.float32)
    awpad[:, :, PAD : PAD + T] = aw
    # X2[b, c*K+k, t] = awpad[b, c, t+k]
    x2 = np.lib.stride_tricks.sliding_window_view(awpad, T, axis=2)
    x2 = np.ascontiguousarray(x2.reshape(B, CK, T))

    weff = Wloc @ Wconv.reshape(N_F, CK)  # [D_ATT, CK]

    import ml_dtypes

    x2 = x2.astype(ml_dtypes.bfloat16)

    base = np.zeros((128, CPK_COLS), dtype=np.float32)
    base[:, OFF_WMT : OFF_WMT + 512] = (
        Wm.T.reshape(4, 128, D_ATT).transpose(1, 0, 2).reshape(128, 512)
    )
    base[0:CK, OFF_WEFFT : OFF_WEFFT + 128] = weff.T
    base[:, OFF_WQT : OFF_WQT + 1024] = (
        Wq.T.reshape(8, 128, D_ATT).transpose(1, 0, 2).reshape(128, 1024)
    )
    base[:, OFF_V] = Wv.reshape(D_ATT)

    memb = memory.astype(ml_dtypes.bfloat16)
    # memn[b, p, n, d] = mem[b, n*128+p, d]; memt[b, p, g, t] = mem[b, t, g*128+p]
    memn_h = memb.reshape(B, 16, 128, D_ENC).transpose(0, 2, 1, 3).reshape(B, 128, 8192)
    memt_h = (
        memb.transpose(0, 2, 1).reshape(B, 4, 128, T).transpose(0, 2, 1, 3).reshape(B, 128, 8192)
    )
    mem2_h = np.ascontiguousarray(np.concatenate([memn_h, memt_h], axis=2))

    cf = np.zeros((128, CF_COLS), dtype=np.float32)
    cf[:, OFF_BQM] = bq + bm
    cf[0:16, OFF_ID16 : OFF_ID16 + 16] = np.eye(16, dtype=np.float32)
    cf[0, OFF_ONES16 : OFF_ONES16 + 16] = 1.0

    in_maps = []
    for i in range(NCORES):
        sl = slice(i * BPC, (i + 1) * BPC)
        cpk = base.copy()
        qTi = query[sl, 0, :].T  # [D_LSTM, BPC]
        cpk[:, OFF_QT : OFF_QT + 64] = (
            qTi.reshape(8, 128, BPC).transpose(1, 0, 2).reshape(128, 64)
        )
        in_maps.append(
            {
                "mem2": mem2_h[sl],
                "x2": x2[sl],
                "cpk": cpk.astype(ml_dtypes.bfloat16),
                "cf32": cf,
            }
        )
    return in_maps


_NC_CACHE = {}


def _get_nc():
    if "nc" not in _NC_CACHE:
        _NC_CACHE["nc"] = build_nc()
    return _NC_CACHE["nc"]


def run(inputs, trace=False, **kw):
    nc = _get_nc()
    in_maps = host_prep(inputs)
    res = run_bass_kernel_spmd(nc, in_maps, list(range(NCORES)), trace=trace, **kw)
    outs = res.results
    ctx = np.concatenate([np.asarray(outs[i]["out_ctx"]) for i in range(NCORES)], axis=0)
    w = np.concatenate([np.asarray(outs[i]["out_w"]) for i in range(NCORES)], axis=0)
    return (ctx, w), res


def kernel(**inputs):
    (ctx, w), _ = run(inputs, trace=False)
    return ctx, w


# revision 25
# speedup vs baseline: 1.2374x; 1.0899x over previous
"""Tacotron2-style location-sensitive attention on 8 TRN2 NeuronCores.

Data-parallel over batch: 64 batches -> 8 per core. Weights replicated.

Math (per batch b):
  q_att = Wq @ query[b] + bq + bm                       [128]
  k     = memory[b] @ Wm.T                              [2048, 128]
  loc.T = WeffT.T @ X2[b]  where Weff = Wloc @ Wconv.reshape(32,62)
  s.T   = k.T + loc.T  (PSUM accumulation), tanh(s + q_att) via ACT bias
  e     = v . tanh_s  (bv dropped: constant shift cancels in softmax)
  w     = softmax(e)  (no max subtraction: |e| <= sum|v| ~ 9, safe in f32)
  ctx   = w @ memory[b]
Outputs: ctx [64, 512], w [64, 2048].

Hardware notes:
 - PE matmul instructions can carry only ONE semaphore wait in this
   toolchain, so all constants arrive via a single DMA (cpk), PSUM
   evacuations are split between DVE/ACT by t-group, and the emission
   order is arranged so recycle-waits are subsumed by earlier waits.
 - All heavy matmuls use float32r (1 col/cycle at N=512 vs 4 for f32).
"""

import os
import sys

sys.path.insert(0, "/opt/trn_rl_repo")

import numpy as np

from concourse import bass, bacc, tile, mybir
from concourse.bass_utils import run_bass_kernel_spmd

B, T = 64, 2048
D_LSTM, D_ENC, D_ATT, N_F, K = 1024, 512, 128, 32, 31
PAD = (K - 1) // 2
NCORES = 8
BPC = B // NCORES  # 8 batches per core
CK = 2 * K  # 62 im2col rows

f32 = mybir.dt.float32
f32r = mybir.dt.float32r
AF = mybir.ActivationFunctionType

bf16 = mybir.dt.bfloat16

# packed bf16 constants cpk [128, CPK_COLS]
OFF_WEFFT = 0  # rows 0..61, 128 cols
OFF_WQT = 128  # [d%128, dch*128+a] 1024 cols
OFF_QT = 1152  # [d%128, dch*8+b] 64 cols
OFF_V = 1216  # 1 col
OFF_WMT = 1217  # [e%128, eg*128+a] 512 cols
CPK_COLS = 1729
# f32 constants cf32 [128, CF_COLS]
OFF_BQM = 0  # 1 col
OFF_ID16 = 1  # rows 0..15, 16 cols
OFF_ONES16 = 17  # row 0, 16 cols
CF_COLS = 33


def build_nc():
    nc = bacc.Bacc(
        "TRN2", target_bir_lowering=False, debug=False, num_devices=NCORES
    )

    memn_d = nc.declare_dram_parameter("memn", [BPC, 128, 16, 512], bf16, isOutput=False)
    memt_d = nc.declare_dram_parameter("memt", [BPC, 128, 4, T], bf16, isOutput=False)
    x2_d = nc.declare_dram_parameter("x2", [BPC, CK, T], bf16, isOutput=False)
    cpk_d = nc.declare_dram_parameter("cpk", [128, CPK_COLS], bf16, isOutput=False)
    cf32_d = nc.declare_dram_parameter("cf32", [128, CF_COLS], f32, isOutput=False)
    ctx_d = nc.declare_dram_parameter("out_ctx", [BPC, D_ENC], f32, isOutput=True)
    w_d = nc.declare_dram_parameter("out_w", [BPC, T], f32, isOutput=True)

    with tile.TileContext(nc) as tc:
        with (
            tc.tile_pool(name="const", bufs=1) as constp,
            tc.tile_pool(name="memn", bufs=3) as memn_p,
            tc.tile_pool(name="memt", bufs=3) as memt_p,
            tc.tile_pool(name="x2p", bufs=2) as x2_p,
            tc.tile_pool(name="work", bufs=3) as work_p,
            tc.tile_pool(name="small", bufs=3) as small_p,
            tc.tile_pool(name="ps_s", bufs=3, space=bass.MemorySpace.PSUM) as ps_s,
            tc.tile_pool(name="ps_e", bufs=2, space=bass.MemorySpace.PSUM) as ps_e,
            tc.tile_pool(name="ps_wt", bufs=1, space=bass.MemorySpace.PSUM) as ps_wt,
            tc.tile_pool(name="ps_cq", bufs=1, space=bass.MemorySpace.PSUM) as ps_cq,
            tc.tile_pool(name="ps_rz", bufs=1, space=bass.MemorySpace.PSUM) as ps_rz,
        ):
            cpk = constp.tile([128, CPK_COLS], bf16, tag="cpk")
            nc.gpsimd.dma_start(cpk[:], cpk_d[:, :])
            cf32 = constp.tile([128, CF_COLS], f32, tag="cf32")
            nc.gpsimd.dma_start(cf32[:], cf32_d[:, :])

            def wmT(eg):
                return cpk[:, OFF_WMT + eg * 128 : OFF_WMT + (eg + 1) * 128]

            weffT = cpk[0:CK, OFF_WEFFT : OFF_WEFFT + 128]

            def wqT(dch):
                return cpk[:, OFF_WQT + dch * 128 : OFF_WQT + (dch + 1) * 128]

            def qTc(dch):
                return cpk[:, OFF_QT + dch * 8 : OFF_QT + (dch + 1) * 8]

            v_ap = cpk[:, OFF_V : OFF_V + 1]
            bqm_ap = cf32[:, OFF_BQM : OFF_BQM + 1]
            id16 = cf32[0:16, OFF_ID16 : OFF_ID16 + 16]
            ones16 = cf32[0:1, OFF_ONES16 : OFF_ONES16 + 16]

            # ---- q_att for all local batches: [a=128, BPC] ----
            qatt_ps = ps_cq.tile([128, BPC], f32, tag="cq_ps")
            for dch in range(8):
                nc.tensor.matmul(
                    qatt_ps[:], wqT(dch), qTc(dch), start=(dch == 0), stop=(dch == 7)
                )
            qatt_sb = constp.tile([128, BPC], f32, tag="qatt")
            nc.scalar.activation(qatt_sb[:], qatt_ps[:], AF.Identity, bias=bqm_ap)

            def stage_b(st):
                """Deferred per-batch tail: wT transpose, context, outputs."""
                b = st["b"]
                exp16_sb = st["exp16"]
                rz_sb = st["rz"]
                memn = st["memn"]
                wT_ps = ps_wt.tile([128, 16], f32, tag="wt_ps")
                nc.tensor.transpose(wT_ps[:], exp16_sb[:], id16)
                wT_sb = work_p.tile([128, 16], bf16, tag="wT")
                nc.scalar.copy(wT_sb[:], wT_ps[:])
                ctx_ps = ps_cq.tile([1, 512], f32, tag="cq_ps")
                for n in range(16):
                    nc.tensor.matmul(
                        ctx_ps[:],
                        wT_sb[:, n : n + 1],
                        memn[:, n, :],
                        start=(n == 0),
                        stop=(n == 15),
                    )
                ctx_sb = work_p.tile([1, 512], f32, tag="ctx_sb")
                nc.scalar.mul(ctx_sb[:], ctx_ps[:], rz_sb[0:1, 0:1])
                nc.scalar.dma_start(ctx_d[b : b + 1, :], ctx_sb[:])
                rz16_ps = ps_rz.tile([16, 1], f32, tag="rz_ps")
                nc.tensor.matmul(rz16_ps[:], ones16, rz_sb[:], start=True, stop=True)
                rz16_sb = small_p.tile([16, 1], f32, tag="rz16")
                nc.scalar.copy(rz16_sb[:], rz16_ps[:])
                w_sb = work_p.tile([16, 128], f32, tag="w_sb")
                nc.scalar.mul(w_sb[:], exp16_sb[:], rz16_sb[:, 0:1])
                nc.scalar.dma_start(w_d[b : b + 1, :], w_sb[:])

            prev = None
            for b in range(BPC):
                # both layouts pre-transposed on host, bf16
                memn = memn_p.tile([128, 16, 512], bf16, tag="memn")
                nc.gpsimd.dma_start(
                    memn[:], memn_d[b : b + 1].rearrange("o p n d -> p (o n) d")
                )
                memt = memt_p.tile([128, 4, T], bf16, tag="memt")
                nc.gpsimd.dma_start(
                    memt[:], memt_d[b : b + 1].rearrange("o p g t -> p (o g) t")
                )
                x2 = x2_p.tile([CK, T], bf16, tag="x2")
                nc.scalar.dma_start(
                    x2[:], x2_d[b : b + 1, :, :].rearrange("o c t -> c (o t)")
                )

                exp_sb = work_p.tile([1, T], f32, tag="exp")
                zs_sb = small_p.tile([1, 4], f32, tag="zs")
                for tg in range(4):
                    tsl = slice(tg * 512, (tg + 1) * 512)
                    # energies for this t-group
                    s_ps = ps_s.tile([128, 512], f32, tag="s_ps")
                    for eg in range(4):
                        nc.tensor.matmul(
                            s_ps[:],
                            wmT(eg),
                            memt[:, eg, tsl],
                            start=(eg == 0),
                            stop=False,
                        )
                    nc.tensor.matmul(
                        s_ps[:], weffT, x2[:, tsl], start=False, stop=True
                    )
                    th_sb = work_p.tile([128, 512], bf16, tag="tanh")
                    nc.scalar.activation(
                        th_sb[:], s_ps[:], AF.Tanh, bias=qatt_sb[:, b : b + 1]
                    )
                    e_ps = ps_e.tile([1, 512], f32, tag="e_ps")
                    nc.tensor.matmul(e_ps[:], v_ap, th_sb[:], start=True, stop=True)
                    nc.scalar.activation(
                        exp_sb[0:1, tsl],
                        e_ps[:],
                        AF.Exp,
                        accum_out=zs_sb[0:1, tg : tg + 1],
                    )

                # softmax denominator (DVE)
                z_sb = small_p.tile([1, 1], f32, tag="z")
                nc.vector.tensor_reduce(
                    z_sb[:], zs_sb[:], axis=mybir.AxisListType.X, op=mybir.AluOpType.add
                )
                rz_sb = small_p.tile([1, 1], f32, tag="rz")
                nc.vector.reciprocal(rz_sb[:], z_sb[:])
                # reshape [1,2048] -> [16,128] via DMA (idle sync ring)
                exp16_sb = work_p.tile([16, 128], f32, tag="exp16")
                nc.sync.dma_start(exp16_sb[:], exp_sb[:])

                if prev is not None:
                    stage_b(prev)
                prev = {"b": b, "exp16": exp16_sb, "rz": rz_sb, "memn": memn}
            stage_b(prev)

    nc.compile()
    return nc


def host_prep(inputs):
    """Precompute constant-folded weights and per-core input shards."""
    query = np.asarray(inputs["query"], dtype=np.float32)  # [B,1,D_LSTM]
    memory = np.ascontiguousarray(np.asarray(inputs["memory"], dtype=np.float32))
    aw = np.asarray(inputs["attention_weights_cat"], dtype=np.float32)  # [B,2,T]
    Wq = np.asarray(inputs["Wq"], dtype=np.float32)
    bq = np.asarray(inputs["bq"], dtype=np.float32)
    Wm = np.asarray(inputs["Wm"], dtype=np.float32)
    bm = np.asarray(inputs["bm"], dtype=np.float32)
    Wconv = np.asarray(inputs["Wconv"], dtype=np.float32)  # [N_F, 2, K]
    Wloc = np.asarray(inputs["Wloc"], dtype=np.float32)  # [D_ATT, N_F]
    Wv = np.asarray(inputs["Wv"], dtype=np.float32)  # [1, D_ATT]

    awpad = np.zeros((B, 2, T + 2 * PAD), dtype=np.float32)
    awpad[:, :, PAD : PAD + T] = aw
    # X2[b, c*K+k, t] = awpad[b, c, t+k]
    x2 = np.lib.stride_tricks.sliding_window_view(awpad, T, axis=2)
    x2 = np.ascontiguousarray(x2.reshape(B, CK, T))

    weff = Wloc @ Wconv.reshape(N_F, CK)  # [D_ATT, CK]

    import ml_dtypes

    x2 = x2.astype(ml_dtypes.bfloat16)

    base = np.zeros((128, CPK_COLS), dtype=np.float32)
    base[:, OFF_WMT : OFF_WMT + 512] = (
        Wm.T.reshape(4, 128, D_ATT).transpose(1, 0, 2).reshape(128, 512)
    )
    base[0:CK, OFF_WEFFT : OFF_WEFFT + 128] = weff.T
    base[:, OFF_WQT : OFF_WQT + 1024] = (
        Wq.T.reshape(8, 128, D_ATT).transpose(1, 0, 2).reshape(128, 1024)
    )
    base[:, OFF_V] = Wv.reshape(D_ATT)

    memb = memory.astype(ml_dtypes.bfloat16)
    # memn[b, p, n, d] = mem[b, n*128+p, d]; memt[b, p, g, t] = mem[b, t, g*128+p]
    memn_h = np.ascontiguousarray(
        memb.reshape(B, 16, 128, D_ENC).transpose(0, 2, 1, 3)
    )
    memt_h = np.ascontiguousarray(
        memb.transpose(0, 2, 1).reshape(B, 4, 128, T).transpose(0, 2, 1, 3)
    )

    cf = np.zeros((128, CF_COLS), dtype=np.float32)
    cf[:, OFF_BQM] = bq + bm
    cf[0:16, OFF_ID16 : OFF_ID16 + 16] = np.eye(16, dtype=np.float32)
    cf[0, OFF_ONES16 : OFF_ONES16 + 16] = 1.0

    in_maps = []
    for i in range(NCORES):
        sl = slice(i * BPC, (i + 1) * BPC)
        cpk = base.copy()
        qTi = query[sl, 0, :].T  # [D_LSTM, BPC]
        cpk[:, OFF_QT : OFF_QT + 64] = (
            qTi.reshape(8, 128, BPC).transpose(1, 0, 2).reshape(128, 64)
        )
        in_maps.append(
            {
                "memn": memn_h[sl],
                "memt": memt_h[sl],
                "x2": x2[sl],
                "cpk": cpk.astype(ml_dtypes.bfloat16),
                "cf32": cf,
            }
        )
    return in_maps


_NC_CACHE = {}


def _get_nc():
    if "nc" not in _NC_CACHE:
        _NC_CACHE["nc"] = build_nc()
    return _NC_CACHE["nc"]


def run(inputs, trace=False, **kw):
    nc = _get_nc()
    in_maps = host_prep(inputs)
    res = run_bass_kernel_spmd(nc, in_maps, list(range(NCORES)), trace=trace, **kw)
    outs = res.results
    ctx = np.concatenate([np.asarray(outs[i]["out_ctx"]) for i in range(NCORES)], axis=0)
    w = np.concatenate([np.asarray(outs[i]["out_w"]) for i in range(NCORES)], axis=0)
    return (ctx, w), res


def kernel(**inputs):
    (ctx, w), _ = run(inputs, trace=False)
    return ctx, w
